# revision 3
# baseline (speedup 1.0000x reference)
"""Trainium2 Bass kernel for MLM tied-weight readout:
    x = embed[ids]; logits = x @ W.T + b; p = softmax(logits); out = p @ W

Strategy (8 NeuronCores, vocab-parallel / tensor-parallel):
  - Host: embedding gather + fp8 cast/transposes (index prep + sharding).
  - Each core owns a 4000-row vocab shard of W/b (padded to 4096).
  - Device, per core, single pass over the vocab shard:
      stage A: L^T[v, m] = (W_c x^T)       (contract h, PE, fp8 DoubleRow)
               usb = S*exp(L^T + b_c)      (ScalarE, PSUM->SBUF fp32)
               u8  = usb - S               (DVE -> fp8; = S*(U-1))
               zacc += usb                 (DVE fp32 running sum over v)
      stage B: Y[m, h] += u8^T @ W_c       (contract v, PE, fp8 DoubleRow)
               Z[m]    = zacc^T @ ones     (tiny fp32 matmul per m-tile)
  - ReduceScatter (sum over cores, scatter over tokens) of [8192, 1025]
    partials; each core divides its token slice by Z and outputs it.

The optional ``reps`` argument loops the whole token pipeline ``reps``
times inside one NEFF (identical data, outputs overwritten in place).
kernel() always uses reps=1; test.py uses a high-reps build to measure
steady-state per-run device time with the axon dispatch latency
amortized away.
"""

import sys

sys.path.insert(0, "/opt/trn_rl_repo")

import functools

import ml_dtypes
import numpy as np

import concourse.bass as bass
import concourse.mybir as mybir
import concourse.tile as tile
from concourse import bacc
from concourse.bass_utils import run_bass_kernel_spmd

BF16 = mybir.dt.bfloat16
FP32 = mybir.dt.float32
FP8 = mybir.dt.float8e4

FP8_SCALE = 16.0                # x,W pre-scale so values exit e4m3 denormals

B, T, H, V = 4, 2048, 1024, 32000
N_CORES = 8
V_SHARD = V // N_CORES          # 4000
V_PAD = 4096                    # padded shard (32 tiles of 128)
NV = V_PAD // 128               # 32 v-tiles per core
NK = H // 128                   # 8 k-tiles (hidden)
M_CHUNK = 512                   # tokens per stage-A chunk
PAD_BIAS = -30.0                # exp(-30) ~ 9e-14: padded rows contribute ~0


def build_program(n_tokens: int, with_rs: bool = True, reps: int = 1):
    """Build the SPMD Bass program for all 8 cores (same code, different data).

    with_rs=False builds a single-core variant (collective replaced by a DMA
    copy) for TimelineSim cost-model profiling.
    """
    n_chunks = n_tokens // M_CHUNK
    mt_per_chunk = M_CHUNK // 128
    tok_shard = n_tokens // N_CORES

    nc = bacc.Bacc(
        "TRN2",
        target_bir_lowering=False,
        debug=False,
        enable_asserts=False,
        num_devices=N_CORES if with_rs else 1,
    )

    xT = nc.dram_tensor("xT", [H, n_tokens], FP8, kind="ExternalInput")
    WT = nc.dram_tensor("WT", [H, V_PAD], FP8, kind="ExternalInput")
    Wn = nc.dram_tensor("Wn", [V_PAD, H], FP8, kind="ExternalInput")
    bia = nc.dram_tensor("bia", [V_PAD, 1], FP32, kind="ExternalInput")
    csum = nc.dram_tensor("csum", [128, H], FP32, kind="ExternalInput")
    out = nc.dram_tensor("out", [tok_shard, H], FP32, kind="ExternalOutput")

    ypart = nc.dram_tensor("ypart", [n_tokens, H + 1], FP32)
    yrs = nc.dram_tensor("yrs", [tok_shard, H + 1], FP32)

    rg = [list(range(N_CORES))]
    # ReduceScatter is issued per row-group of RS_GROUP token rows so the
    # collective (TOPSP/SDMA) overlaps with the remaining PE compute. Rank c
    # receives rows [g*RS_GROUP + c*RS_OUT, +RS_OUT) -> out row block g.
    RS_GROUP = 1024
    RS_OUT = RS_GROUP // N_CORES                     # 128
    chunks_per_group = RS_GROUP // M_CHUNK           # 2

    with tile.TileContext(nc) as tc:
        with (
            tc.tile_pool(name="wn_res", bufs=1) as wn_pool,
            tc.tile_pool(name="wt_res", bufs=1) as wt_pool,
            tc.tile_pool(name="const", bufs=1) as const_pool,
            tc.tile_pool(name="xt", bufs=2) as xt_pool,
            tc.tile_pool(name="ut", bufs=1) as ut_pool,
            tc.tile_pool(name="usb", bufs=3) as usb_pool,
            tc.tile_pool(name="zacc", bufs=2) as zacc_pool,
            tc.tile_pool(name="ysb", bufs=2) as ysb_pool,
            tc.tile_pool(name="zsb", bufs=2) as zsb_pool,
            tc.tile_pool(name="fin", bufs=2) as fin_pool,
            tc.tile_pool(name="psA", bufs=2, space="PSUM") as psA_pool,
            tc.tile_pool(name="psY", bufs=2, space="PSUM") as psY_pool,
            tc.tile_pool(name="psZ", bufs=2, space="PSUM") as psZ_pool,
        ):
            # --- resident weights (wt first: stage A needs it immediately) ---
            wt = []
            # DoubleRow: tile[p, i*V_PAD + v] = WT[k*256 + i*128 + p, v]
            for k in range(NK // 2):
                t = wt_pool.tile([128, 2 * V_PAD], FP8, tag=f"wt{k}")
                nc.sync.dma_start(t[:, 0:V_PAD], WT[k * 256 : k * 256 + 128, :])
                nc.sync.dma_start(
                    t[:, V_PAD : 2 * V_PAD], WT[k * 256 + 128 : k * 256 + 256, :]
                )
                wt.append(t)
            wn = []
            # v-pair tiles: tile[p, i*H + h] = Wn[j*256 + i*128 + p, h]
            for j in range(NV // 2):
                t = wn_pool.tile([128, 2 * H], FP8, tag=f"wn{j}")
                nc.sync.dma_start(t[:, 0:H], Wn[j * 256 : j * 256 + 128, :])
                nc.sync.dma_start(
                    t[:, H : 2 * H], Wn[j * 256 + 128 : j * 256 + 256, :]
                )
                wn.append(t)
            btile = const_pool.tile([128, NV], FP32, tag="btile")
            for v in range(NV):
                nc.sync.dma_start(
                    btile[:, v : v + 1], bia[v * 128 : (v + 1) * 128, :]
                )
            onesf = const_pool.tile([128, 1], FP32, tag="onesf")
            nc.vector.memset(onesf[:], 1.0)
            cs_tile = const_pool.tile([128, H], FP32, tag="cs")
            nc.sync.dma_start(cs_tile[:], csum[:])

            # --- main pipeline over token chunks ---
            for rep in range(reps):
              for c in range(n_chunks):
                m0 = c * M_CHUNK
                xts = []
                for k in range(NK // 2):
                    t = xt_pool.tile([128, 2 * M_CHUNK], FP8, tag=f"xt{k}")
                    nc.sync.dma_start(
                        t[:, 0:M_CHUNK],
                        xT[k * 256 : k * 256 + 128, m0 : m0 + M_CHUNK],
                    )
                    nc.sync.dma_start(
                        t[:, M_CHUNK : 2 * M_CHUNK],
                        xT[k * 256 + 128 : k * 256 + 256, m0 : m0 + M_CHUNK],
                    )
                    xts.append(t)
                # stage A: usb = S*exp(W_c x^T + b); u8 = usb - S; zacc += usb
                ut = []
                zacc = zacc_pool.tile([128, M_CHUNK], FP32, tag="zacc")
                for v in range(NV):
                    pA = psA_pool.tile([128, M_CHUNK], FP32, tag="pA")
                    for k in range(NK // 2):
                        lhs3 = wt[k][:].rearrange("p (two v) -> p two v", two=2)
                        rhs3 = xts[k][:].rearrange("p (two m) -> p two m", two=2)
                        nc.tensor.matmul(
                            pA[:],
                            lhsT=lhs3[:, :, v * 128 : (v + 1) * 128],
                            rhs=rhs3,
                            start=(k == 0),
                            stop=(k == NK // 2 - 1),
                            perf_mode=mybir.MatmulPerfMode.DoubleRow,
                        )
                    sA = 1.0 / (FP8_SCALE * FP8_SCALE)
                    # ACT emits S*exp(L+b) in fp32 (bias pre-folded with
                    # ln S on host); DVE subtracts S -> u8 = S*(U-1) fp8.
                    if v % 2 == 0:
                        up = ut_pool.tile([128, 2 * M_CHUNK], FP8, tag=f"ut{v // 2}")
                        ut.append(up)
                    usb = usb_pool.tile([128, M_CHUNK], FP32, tag="usb")
                    nc.scalar.activation(
                        usb[:],
                        pA[:],
                        mybir.ActivationFunctionType.Exp,
                        bias=btile[:, v : v + 1],
                        scale=sA,
                    )
                    half = v % 2
                    nc.vector.tensor_scalar_add(
                        ut[v // 2][:, half * M_CHUNK : (half + 1) * M_CHUNK],
                        usb[:],
                        -FP8_SCALE,
                    )
                    if v == 0:
                        nc.vector.tensor_copy(zacc[:], usb[:])
                    else:
                        nc.vector.tensor_add(zacc[:], zacc[:], usb[:])
                # stage B: Y[m, h] = u8 W_c ; Z[m] = sum_p zacc[p, m]
                for mt in range(mt_per_chunk):
                    pY = psY_pool.tile([128, H], FP32, tag="pY")
                    for j in range(NV // 2):
                        lhs3 = ut[j][:].rearrange("p (two m) -> p two m", two=2)[
                            :, :, mt * 128 : (mt + 1) * 128
                        ]
                        rhs3 = wn[j][:].rearrange("p (two h) -> p two h", two=2)
                        st, sp = (j == 0), (j == NV // 2 - 1)
                        nc.tensor.matmul(
                            pY[:, 0:512], lhsT=lhs3, rhs=rhs3[:, :, 0:512],
                            start=st, stop=sp,
                            perf_mode=mybir.MatmulPerfMode.DoubleRow,
                        )
                        nc.tensor.matmul(
                            pY[:, 512:1024], lhsT=lhs3, rhs=rhs3[:, :, 512:1024],
                            start=st, stop=sp,
                            perf_mode=mybir.MatmulPerfMode.DoubleRow,
                        )
                    pZ = psZ_pool.tile([128, 1], FP32, tag="pZ")
                    nc.tensor.matmul(
                        pZ[:],
                        lhsT=zacc[:, mt * 128 : (mt + 1) * 128],
                        rhs=onesf[:, 0:1],
                        start=True, stop=True,
                    )
                    ysb = ysb_pool.tile([128, H], FP32, tag="ysb")
                    nc.vector.tensor_copy(ysb[:], pY[:])
                    zsb = zsb_pool.tile([128, 1], FP32, tag="zsb")
                    nc.vector.tensor_copy(zsb[:], pZ[:])
                    r0 = m0 + mt * 128
                    nc.sync.dma_start(ypart[r0 : r0 + 128, 0:H], ysb[:])
                    nc.sync.dma_start(ypart[r0 : r0 + 128, H : H + 1], zsb[:])

                # --- per-row-group: reduce partials over cores + divide ---
                if (c + 1) % chunks_per_group == 0:
                    g = c // chunks_per_group
                    g0 = g * RS_GROUP
                    o0 = g * RS_OUT
                    if with_rs:
                        nc.gpsimd.collective_compute(
                            "ReduceScatter",
                            mybir.AluOpType.add,
                            replica_groups=rg,
                            ins=[ypart[g0 : g0 + RS_GROUP, :]],
                            outs=[yrs[o0 : o0 + RS_OUT, :]],
                        )
                    else:
                        nc.sync.dma_start(
                            yrs[o0 : o0 + RS_OUT, :], ypart[g0 : g0 + RS_OUT, :]
                        )
                    yt = fin_pool.tile([128, H + 1], FP32, tag="yt")
                    nc.sync.dma_start(yt[:], yrs[o0 : o0 + RS_OUT, :])
                    # Z column arrives as S*Z_tot; numerator as S^2*(Y - 1@W)
                    zt = fin_pool.tile([128, 1], FP32, tag="zt")
                    nc.vector.tensor_scalar_mul(
                        zt[:], yt[:, H : H + 1], 1.0 / FP8_SCALE
                    )
                    zinv = fin_pool.tile([128, 1], FP32, tag="zinv")
                    nc.vector.reciprocal(zinv[:], zt[:])
                    ysc = fin_pool.tile([128, H], FP32, tag="ysc")
                    nc.vector.tensor_scalar_mul(
                        ysc[:], yt[:, 0:H], 1.0 / (FP8_SCALE * FP8_SCALE)
                    )
                    nc.vector.tensor_add(ysc[:], ysc[:], cs_tile[:])
                    ot = fin_pool.tile([128, H], FP32, tag="ot")
                    nc.scalar.mul(ot[:], ysc[:, 0:H], mul=zinv[:, 0:1])
                    nc.sync.dma_start(out[o0 : o0 + RS_OUT, :], ot[:])

    nc.compile()
    return nc


@functools.lru_cache(maxsize=2)
def _cached_program(n_tokens: int):
    return build_program(n_tokens)


def prep_inputs(input_ids, embed_table, W, b, n_tokens=None):
    """Host-side sharding/prep: gather, cast to fp8, transpose, pad."""
    ids = np.asarray(input_ids).reshape(-1).astype(np.int64)
    if n_tokens is not None:
        ids = ids[:n_tokens]
    embed = np.ascontiguousarray(np.asarray(embed_table, dtype=np.float32))
    W = np.ascontiguousarray(np.asarray(W, dtype=np.float32))
    b = np.asarray(b, dtype=np.float32).reshape(-1)

    f8 = ml_dtypes.float8_e4m3
    x = embed[ids]                                   # [n_tok, H] fp32
    xT = np.ascontiguousarray(x.T * FP8_SCALE).astype(f8)      # [H, n_tok]

    csum = np.broadcast_to(W.sum(axis=0, dtype=np.float64).astype(np.float32), (128, H))
    csum = np.ascontiguousarray(csum)
    in_maps = []
    for c in range(N_CORES):
        lo = c * V_SHARD
        Wc = W[lo : lo + V_SHARD]                    # [4000, H]
        Wn_c = np.zeros((V_PAD, H), dtype=f8)
        Wn_c[:V_SHARD] = (Wc * FP8_SCALE).astype(f8)
        WT_c = np.zeros((H, V_PAD), dtype=f8)
        WT_c[:, :V_SHARD] = np.ascontiguousarray(Wc.T * FP8_SCALE).astype(f8)
        b_c = np.full((V_PAD, 1), PAD_BIAS, dtype=np.float32)
        b_c[:V_SHARD, 0] = b[lo : lo + V_SHARD]
        # ACT emits S*exp(L+b) directly: fold ln S into the bias
        b_c += np.log(FP8_SCALE)
        m = {"xT": xT, "WT": WT_c, "Wn": Wn_c, "bia": b_c, "csum": csum}
        in_maps.append(m)
    return in_maps


def run(inputs, n_tokens=B * T, **spmd_kwargs):
    nc = _cached_program(n_tokens)
    in_maps = prep_inputs(
        inputs["input_ids"], inputs["embed_table"], inputs["W"], inputs["b"],
        n_tokens=n_tokens,
    )
    res = run_bass_kernel_spmd(nc, in_maps, core_ids=list(range(N_CORES)), **spmd_kwargs)
    full = unshard([res.results[c]["out"] for c in range(N_CORES)], n_tokens)
    return full, res


def unshard(parts, n_tokens):
    # rank c's output rows are [g*1024 + c*128, +128) for each row-group g
    n_groups = n_tokens // 1024
    arr = np.stack([np.asarray(p).reshape(n_groups, 128, H) for p in parts], axis=1)
    return arr.reshape(n_tokens, H)                  # [n_tokens, H] fp32


def kernel(input_ids, embed_table, W, b):
    full, _ = run(
        {"input_ids": input_ids, "embed_table": embed_table, "W": W, "b": b}
    )
    return full.reshape(B, T, H).astype(np.float32)
